# revision 18
# baseline (speedup 1.0000x reference)
"""Causal self-attention on 8 TRN2 NeuronCores.

Sharding: core c handles batch b = c//2 and head-group g = c%2 (8 of 16 heads).
Each core computes its partial y^T = w_proj[slice].T @ o^T (contraction over its
512 o-channels); the host sums the two partials per batch and adds b_proj.

Shapes (hardcoded): B=4, T=2048, C=1024, H=16, HD=64.
"""

import numpy as np

B, T, C, H = 4, 2048, 1024, 16
HD = C // H          # 64
G = 2                # head groups
NHL = H // G         # 8 heads per core
GQ = NHL * HD        # 512 channel slice per core
P = 128
NT = T // P          # 16 token tiles / k-chunks
NCHUNK = C // P      # 8 contraction chunks for qkv
SCALE = 1.0 / float(np.sqrt(HD))

_PROGRAM = None


def _emit(ctx, tc, aps, mybir, bass):
    import contextlib

    nc = tc.nc
    f32 = mybir.dt.float32
    f32r = mybir.dt.float32r
    bf16 = mybir.dt.bfloat16
    EXP = mybir.ActivationFunctionType.Exp

    x_d, wqkv_d, bqk_d, bv_d, wp_d, yT_d = (
        aps["x"], aps["wqkv"], aps["bqk"], aps["bv"], aps["wp"], aps["yT"],
    )

    # ---------------- pools ----------------
    const = ctx.enter_context(tc.tile_pool(name="const", bufs=1))
    dramp = ctx.enter_context(tc.tile_pool(name="dramp", bufs=1, space="DRAM"))
    # psum: 2 + 4 + 2 = 8 banks
    ps_ab = ctx.enter_context(tc.tile_pool(name="ps_ab", bufs=2, space="PSUM"))
    ps_sc = ctx.enter_context(tc.tile_pool(name="ps_sc", bufs=2, space="PSUM"))
    ps_pv = ctx.enter_context(tc.tile_pool(name="ps_pv", bufs=2, space="PSUM"))

    qkp = ctx.enter_context(tc.tile_pool(name="qkp", bufs=8))
    vap = ctx.enter_context(tc.tile_pool(name="vap", bufs=16))
    ptp = ctx.enter_context(tc.tile_pool(name="ptp", bufs=2))
    otp = ctx.enter_context(tc.tile_pool(name="otp", bufs=2))
    rcp = ctx.enter_context(tc.tile_pool(name="rcp", bufs=1))

    # constants
    identity = const.tile([P, P], f32)
    from concourse.masks import make_identity
    make_identity(nc, identity)
    bqk_sb = const.tile([P, 8], f32)
    nc.sync.dma_start(bqk_sb[:], bqk_d[:])
    bvb = const.tile([P, GQ], f32)
    nc.sync.dma_start(bvb[:], bv_d[None, :].to_broadcast((P, GQ)))
    ones8 = const.tile([P, NHL, 1], f32)
    nc.vector.memset(ones8[:], 1.0)

    odram = dramp.tile([GQ, T], f32r, space="DRAM")

    # ---------------- phase A: load x, build xT ----------------
    stackAB = contextlib.ExitStack()
    xTp = stackAB.enter_context(tc.tile_pool(name="xTp", bufs=8))
    wqkp = stackAB.enter_context(tc.tile_pool(name="wqkp", bufs=8))
    wvp = stackAB.enter_context(tc.tile_pool(name="wvp", bufs=1))
    stackA = contextlib.ExitStack()
    xp = stackA.enter_context(tc.tile_pool(name="xp", bufs=2))

    xT = []  # 8 tiles [128 c, 2048 t] f32
    for r in range(NCHUNK):
        t_ = xTp.tile([P, T], f32r, name=f"xT{r}", tag="xT")
        xT.append(t_)

    # x tile t -> for each r, transpose block into psum, 2 t-blocks per psum tile
    for tg in range(NT // 2):  # groups of 2 t-tiles
        xts = []
        for tt in range(2):
            t = 2 * tg + tt
            x_t = xp.tile([P, C], f32, name=f"x_{t}", tag="x")
            nc.sync.dma_start(x_t[:], x_d[t * P:(t + 1) * P, :])
            xts.append(x_t)
        for r in range(NCHUNK):
            tp = ps_ab.tile([P, 256], f32, name=f"tp_{tg}_{r}", tag="ps_ab")
            for tt in range(2):
                nc.tensor.transpose(
                    tp[:, tt * P:(tt + 1) * P],
                    xts[tt][:, r * P:(r + 1) * P],
                    identity,
                )
            nc.vector.tensor_copy(xT[r][:, tg * 256:(tg + 1) * 256], tp[:])
    stackA.close()

    # ---------------- phase B: qkv ----------------
    # q/k transposed: for ct in 0..7 (4 q-tiles then 4 k-tiles),
    # out tile [128 c', 2048 t] accumulating 8 chunks, 4 t-windows of 512.
    qkT = []  # bf16 tiles; 0..3 = qT, 4..7 = kT
    for ct in range(8):
        o_t = qkp.tile([P, T], bf16, name=f"qkT{ct}", tag="qkT")
        qkT.append(o_t)

    # v natural + ones col: vaug[t-tile] = [128 t, 8 heads, 65]
    vaug = []
    for t in range(NT):
        va = vap.tile([P, NHL, HD + 1], f32r, name=f"vaug{t}", tag="vaug")
        nc.vector.tensor_copy(va[:, :, HD:HD + 1], ones8[:])
        vaug.append(va)

    # emission order: q0,k0 first so attention can start early, then v, then rest
    wqkv_r = wqkv_d.rearrange("(a p) n -> p a n", p=P)  # [128, 8, 1536]

    def emit_qk_tile(ct):
        # ct in 0..7 -> col range in wqkv slice: q tiles 0..3 -> cols 128*ct,
        # k tiles -> 512 + 128*(ct-4); source layout is [q(512) k(512) v(512)]
        col0 = ct * P
        w_t = wqkp.tile([P, NCHUNK, P], f32r, name=f"wqk_{ct}", tag="wqk")
        nc.sync.dma_start(w_t[:], wqkv_r[:, :, col0:col0 + P])
        for tw in range(4):
            ps = ps_ab.tile([P, 512], f32, name=f"qkps_{ct}_{tw}", tag="ps_ab")
            for a in range(NCHUNK):
                nc.tensor.matmul(
                    ps[:],
                    w_t[:, a, :],
                    xT[a][:, tw * 512:(tw + 1) * 512],
                    start=(a == 0),
                    stop=(a == NCHUNK - 1),
                )
            # add bias (per-partition) and cast to bf16
            nc.vector.tensor_scalar_add(
                qkT[ct][:, tw * 512:(tw + 1) * 512], ps[:], bqk_sb[:, ct:ct + 1]
            )

    def emit_v():
        w_t = wvp.tile([P, NCHUNK, GQ], f32r, name="wv", tag="wv")
        nc.sync.dma_start(w_t[:], wqkv_r[:, :, 2 * GQ:3 * GQ])
        for t in range(NT):
            ps = ps_ab.tile([P, GQ], f32, name=f"vps_{t}", tag="ps_ab")
            for a in range(NCHUNK):
                nc.tensor.matmul(
                    ps[:],
                    xT[a][:, t * P:(t + 1) * P],
                    w_t[:, a, :],
                    start=(a == 0),
                    stop=(a == NCHUNK - 1),
                )
            nc.vector.tensor_add(
                vaug[t][:, :, 0:HD],
                ps[:].rearrange("p (h d) -> p h d", h=NHL),
                bvb[:].rearrange("p (h d) -> p h d", h=NHL),
            )

    emit_qk_tile(0)  # q tile 0
    emit_qk_tile(4)  # k tile 0
    emit_v()
    for j in range(1, 4):
        emit_qk_tile(j)
        emit_qk_tile(4 + j)

    # ---------------- phase C: attention ----------------
    # head pairs interleaved: head A on partition strip 0, head B on strip 64
    # (row-packed score matmuls run concurrently in the PE array).
    def act_recip(out, in_):
        # nc.scalar.activation refuses Reciprocal outright; emit the
        # InstActivation directly (denominators are well-conditioned sums
        # of positives; measured error is checked by the test harness).
        eng = nc.scalar
        ins_ = [eng.lower_ap(in_)]
        for v in (0.0, 1.0, 0.0):  # bias, scale, alpha
            ins_.append(mybir.ImmediateValue(dtype=mybir.dt.float32, value=v))
        return eng.add_instruction(
            mybir.InstActivation(
                name=eng.bass.get_next_instruction_name(),
                func=mybir.ActivationFunctionType.Reciprocal,
                ins=ins_,
                outs=[eng.lower_ap(out)],
            )
        )
    for hp in range(4):
        qt = qkT[hp]
        kt = qkT[4 + hp]
        for half in range(2):
            qlo = half * 1024
            qhi = qlo + 1024
            nch = qhi // P  # chunks in this half
            pv = []
            for hh in range(2):
                pvt = ps_pv.tile([P, 1024], f32, name=f"pv_{hp}_{half}_{hh}",
                                 tag="ps_pv")
                pv.append(pvt)
            for i in range(nch):
                qs = max(qlo, i * P)
                for pw in range(2):
                    ws = qlo + pw * 512
                    we = ws + 512
                    s = max(qs, ws)
                    if s >= we:
                        continue
                    for hh in range(2):
                        h = 2 * hp + hh
                        r0 = hh * HD
                        sc = ps_sc.tile([P, 512], f32,
                                        name=f"sc_{h}_{half}_{i}_{pw}", tag="ps_sc")
                        nc.tensor.matmul(
                            sc[:, s - ws:],
                            kt[r0:r0 + HD, i * P:(i + 1) * P],
                            qt[r0:r0 + HD, s:we],
                            start=True,
                            stop=True,
                        )
                        pt = ptp.tile([P, 512], f32r,
                                      name=f"pt_{h}_{half}_{i}_{pw}", tag="pt")
                        nc.scalar.activation(pt[:, s - ws:], sc[:, s - ws:],
                                             EXP, scale=SCALE)
                        if ws <= i * P and i * P + P <= we:
                            o = i * P - ws
                            nc.gpsimd.affine_select(
                                out=pt[:, o:o + P],
                                in_=pt[:, o:o + P],
                                compare_op=mybir.AluOpType.is_ge,
                                fill=0.0,
                                base=0,
                                pattern=[[1, P]],
                                channel_multiplier=-1,
                            )
                        nc.tensor.matmul(
                            pv[hh][0:HD + 1, pw * 512 + s - ws:pw * 512 + 512],
                            vaug[i][:, h, :],
                            pt[:, s - ws:],
                            start=(i == 0),
                            stop=(i == we // P - 1),
                        )
            # normalize both heads; write shared [128, 1024] tile, one DMA
            ot = otp.tile([P, 1024], f32r, name=f"ot_{hp}_{half}", tag="ot")
            for hh in range(2):
                rc = rcp.tile([1, 1024], f32, name=f"rc_{hp}_{half}_{hh}", tag="rc")
                act_recip(rc[:], pv[hh][HD:HD + 1, :])
                rcb = rcp.tile([HD, 1024], f32, name=f"rcb_{hp}_{half}_{hh}",
                               tag="rcb")
                nc.gpsimd.partition_broadcast(rcb[:], rc[:])
                nc.vector.tensor_mul(ot[hh * HD:(hh + 1) * HD, :],
                                     pv[hh][0:HD, :], rcb[:])
            nc.sync.dma_start(odram[hp * P:(hp + 1) * P, qlo:qhi], ot[:])

    stackAB.close()  # release x / w pools (xT stays for LIFO ordering)

    # ---------------- phase D: proj ----------------
    stackD = contextlib.ExitStack()
    orp = stackD.enter_context(tc.tile_pool(name="orp", bufs=4))
    wpp = stackD.enter_context(tc.tile_pool(name="wpp", bufs=1))
    ysp = stackD.enter_context(tc.tile_pool(name="ysp", bufs=3))

    wp_t = wpp.tile([P, 4, C], f32r, name="wp", tag="wp")
    nc.sync.dma_start(wp_t[:], wp_d.rearrange("(a p) n -> p a n", p=P))
    oTr = []
    for a in range(4):
        o_t = orp.tile([P, T], f32r, name=f"oTr{a}", tag="oTr")
        nc.sync.dma_start(o_t[:], odram[a * P:(a + 1) * P, :])
        oTr.append(o_t)
    for m in range(NCHUNK):  # cout tiles
        for tw in range(4):
            ps = ps_ab.tile([P, 512], f32, name=f"yps_{m}_{tw}", tag="ps_ab")
            for a in range(4):
                nc.tensor.matmul(
                    ps[:],
                    wp_t[:, a, m * P:(m + 1) * P],
                    oTr[a][:, tw * 512:(tw + 1) * 512],
                    start=(a == 0),
                    stop=(a == 3),
                )
            ys = ysp.tile([P, 512], f32, name=f"ys_{m}_{tw}", tag="ys")
            nc.scalar.copy(ys[:], ps[:])
            nc.sync.dma_start(
                yT_d[m * P:(m + 1) * P, tw * 512:(tw + 1) * 512], ys[:]
            )
    stackD.close()


def _build_program():
    import contextlib

    import concourse.bass as bass
    import concourse.mybir as mybir
    import concourse.tile as tile
    from concourse import bacc

    nc = bacc.Bacc("TRN2", target_bir_lowering=False, debug=False, num_devices=8)
    f32 = mybir.dt.float32
    aps = {
        "x": nc.dram_tensor("x", [T, C], f32, kind="ExternalInput").ap(),
        "wqkv": nc.dram_tensor("wqkv", [C, 3 * GQ], mybir.dt.float32r, kind="ExternalInput").ap(),
        "bqk": nc.dram_tensor("bqk", [P, 8], f32, kind="ExternalInput").ap(),
        "bv": nc.dram_tensor("bv", [GQ], f32, kind="ExternalInput").ap(),
        "wp": nc.dram_tensor("wp", [GQ, C], mybir.dt.float32r, kind="ExternalInput").ap(),
        "yT": nc.dram_tensor("yT", [C, T], f32, kind="ExternalOutput").ap(),
    }
    with tile.TileContext(nc) as tc:
        with contextlib.ExitStack() as ctx:
            _emit(ctx, tc, aps, mybir, bass)
    nc.compile()
    return nc


def get_program():
    global _PROGRAM
    if _PROGRAM is None:
        _PROGRAM = _build_program()
    return _PROGRAM


def make_in_maps(x, w_qkv, b_qkv, w_proj):
    x = np.asarray(x, np.float32)
    w_qkv = np.asarray(w_qkv, np.float32)
    b_qkv = np.asarray(b_qkv, np.float32)
    w_proj = np.asarray(w_proj, np.float32)
    in_maps = []
    for c in range(8):
        b = c // 2
        g = c % 2
        q0 = g * GQ
        wq = w_qkv[:, q0:q0 + GQ]
        wk = w_qkv[:, C + q0:C + q0 + GQ]
        wv = w_qkv[:, 2 * C + q0:2 * C + q0 + GQ]
        wqkv = np.ascontiguousarray(np.concatenate([wq, wk, wv], axis=1))
        bq = b_qkv[q0:q0 + GQ]
        bk = b_qkv[C + q0:C + q0 + GQ]
        bqk = np.ascontiguousarray(np.concatenate([bq, bk]).reshape(8, P).T)
        bv = np.ascontiguousarray(b_qkv[2 * C + q0:2 * C + q0 + GQ])
        in_maps.append({
            "x": np.ascontiguousarray(x[b]),
            "wqkv": wqkv,
            "bqk": bqk,
            "bv": bv,
            "wp": np.ascontiguousarray(w_proj[q0:q0 + GQ, :]),
        })
    return in_maps


def combine_outputs(outs, b_proj):
    b_proj = np.asarray(b_proj, np.float32)
    y = np.empty((B, T, C), np.float32)
    for b in range(B):
        acc = outs[2 * b] + outs[2 * b + 1]  # [C, T]
        y[b] = acc.T + b_proj
    return y


def kernel(x, w_qkv, b_qkv, w_proj, b_proj, _trace=False):
    from concourse import bass_utils

    nc = get_program()
    in_maps = make_in_maps(x, w_qkv, b_qkv, w_proj)
    res = bass_utils.run_bass_kernel_spmd(
        nc, in_maps, core_ids=list(range(8)), trace=_trace
    )
    outs = [r["yT"] for r in res.results]
    y = combine_outputs(outs, b_proj)
    if _trace:
        return y, res
    return y


# revision 21
# speedup vs baseline: 1.2678x; 1.2678x over previous
"""Causal self-attention on 8 TRN2 NeuronCores.

Sharding: core c handles batch b = c//2 and head-group g = c%2 (8 of 16 heads).
Each core computes its partial y^T = w_proj[slice].T @ o^T (contraction over its
512 o-channels); the host sums the two partials per batch and adds b_proj.

Shapes (hardcoded): B=4, T=2048, C=1024, H=16, HD=64.
"""

import numpy as np

B, T, C, H = 4, 2048, 1024, 16
HD = C // H          # 64
G = 2                # head groups
NHL = H // G         # 8 heads per core
GQ = NHL * HD        # 512 channel slice per core
P = 128
NT = T // P          # 16 token tiles / k-chunks
NCHUNK = C // P      # 8 contraction chunks for qkv
SCALE = 1.0 / float(np.sqrt(HD))

_PROGRAM = None


def _emit(ctx, tc, aps, mybir, bass):
    import contextlib

    nc = tc.nc
    f32 = mybir.dt.float32
    f32r = mybir.dt.float32r
    bf16 = mybir.dt.bfloat16
    EXP = mybir.ActivationFunctionType.Exp

    x_d, wqkv_d, bqk_d, bv_d, wp_d, yT_d = (
        aps["x"], aps["wqkv"], aps["bqk"], aps["bv"], aps["wp"], aps["yT"],
    )

    # ---------------- pools ----------------
    const = ctx.enter_context(tc.tile_pool(name="const", bufs=1))
    dramp = ctx.enter_context(tc.tile_pool(name="dramp", bufs=1, space="DRAM"))
    # psum: main 2x[128,1024] (4 banks) + pv 4x[128,512] (4 banks)
    ps_main = ctx.enter_context(tc.tile_pool(name="ps_main", bufs=2, space="PSUM"))
    ps_pv = ctx.enter_context(tc.tile_pool(name="ps_pv", bufs=4, space="PSUM"))

    qkp = ctx.enter_context(tc.tile_pool(name="qkp", bufs=8))
    vap = ctx.enter_context(tc.tile_pool(name="vap", bufs=16))
    ptp = ctx.enter_context(tc.tile_pool(name="ptp", bufs=2))
    otp = ctx.enter_context(tc.tile_pool(name="otp", bufs=2))
    rcp = ctx.enter_context(tc.tile_pool(name="rcp", bufs=2))

    # constants
    identity = const.tile([P, P], f32)
    from concourse.masks import make_identity
    make_identity(nc, identity)
    bqk_sb = const.tile([P, 8], f32)
    nc.sync.dma_start(bqk_sb[:], bqk_d[:])
    bvb = const.tile([P, GQ], f32)
    nc.sync.dma_start(bvb[:], bv_d[None, :].to_broadcast((P, GQ)))
    ones8 = const.tile([P, NHL, 1], f32)
    nc.vector.memset(ones8[:], 1.0)

    odram = dramp.tile([GQ, T], f32r, space="DRAM")

    # ---------------- phase A: load x, build xT ----------------
    stackAB = contextlib.ExitStack()
    xTp = stackAB.enter_context(tc.tile_pool(name="xTp", bufs=8))
    wqkp = stackAB.enter_context(tc.tile_pool(name="wqkp", bufs=4))
    wvp = stackAB.enter_context(tc.tile_pool(name="wvp", bufs=1))
    stackA = contextlib.ExitStack()
    xp = stackA.enter_context(tc.tile_pool(name="xp", bufs=2))

    xT = []  # 8 tiles [128 c, 2048 t]
    for r in range(NCHUNK):
        t_ = xTp.tile([P, T], f32r, name=f"xT{r}", tag="xT")
        xT.append(t_)

    for tg in range(NT // 2):  # groups of 2 t-tiles
        xts = []
        for tt in range(2):
            t = 2 * tg + tt
            x_t = xp.tile([P, C], f32, name=f"x_{t}", tag="x")
            nc.sync.dma_start(x_t[:], x_d[t * P:(t + 1) * P, :])
            xts.append(x_t)
        for r in range(NCHUNK):
            tp = ps_main.tile([P, 256], f32, name=f"tp_{tg}_{r}", tag="main")
            for tt in range(2):
                nc.tensor.transpose(
                    tp[:, tt * P:(tt + 1) * P],
                    xts[tt][:, r * P:(r + 1) * P],
                    identity,
                )
            nc.vector.tensor_copy(xT[r][:, tg * 256:(tg + 1) * 256], tp[:])
    stackA.close()

    # ---------------- phase B: qkv ----------------
    qkT = []  # bf16 tiles [128 c', 2048 t]; 0..3 = qT, 4..7 = kT
    for ct in range(8):
        o_t = qkp.tile([P, T], bf16, name=f"qkT{ct}", tag="qkT")
        qkT.append(o_t)

    vaug = []  # [128 k, 8 heads, 65] per k-chunk (col 64 = ones for denom)
    for t in range(NT):
        va = vap.tile([P, NHL, HD + 1], f32r, name=f"vaug{t}", tag="vaug")
        nc.vector.tensor_copy(va[:, :, HD:HD + 1], ones8[:])
        vaug.append(va)

    wqkv_r = wqkv_d.rearrange("(a p) n -> p a n", p=P)  # [128, 8, 1536]

    def emit_qk_tile(ct):
        col0 = ct * P
        w_t = wqkp.tile([P, NCHUNK, P], f32r, name=f"wqk_{ct}", tag="wqk")
        nc.sync.dma_start(w_t[:], wqkv_r[:, :, col0:col0 + P])
        for twp in range(2):  # two [*,1024] psum tiles per output tile
            ps = ps_main.tile([P, 1024], f32, name=f"qkps_{ct}_{twp}", tag="main")
            for a in range(NCHUNK):
                for sw in range(2):
                    nc.tensor.matmul(
                        ps[:, sw * 512:(sw + 1) * 512],
                        w_t[:, a, :],
                        xT[a][:, twp * 1024 + sw * 512:twp * 1024 + (sw + 1) * 512],
                        start=(a == 0),
                        stop=(a == NCHUNK - 1),
                    )
            nc.vector.tensor_scalar_add(
                qkT[ct][:, twp * 1024:(twp + 1) * 1024], ps[:], bqk_sb[:, ct:ct + 1]
            )

    def emit_v():
        w_t = wvp.tile([P, NCHUNK, GQ], f32r, name="wv", tag="wv")
        nc.sync.dma_start(w_t[:], wqkv_r[:, :, 2 * GQ:3 * GQ])
        for t in range(NT):
            ps = ps_pv.tile([P, GQ], f32, name=f"vps_{t}", tag="ps_pv")
            for a in range(NCHUNK):
                nc.tensor.matmul(
                    ps[:],
                    xT[a][:, t * P:(t + 1) * P],
                    w_t[:, a, :],
                    start=(a == 0),
                    stop=(a == NCHUNK - 1),
                )
            nc.vector.tensor_add(
                vaug[t][:, :, 0:HD],
                ps[:].rearrange("p (h d) -> p h d", h=NHL),
                bvb[:].rearrange("p (h d) -> p h d", h=NHL),
            )

    emit_qk_tile(0)
    emit_qk_tile(4)
    emit_v()
    for j in range(1, 4):
        emit_qk_tile(j)
        emit_qk_tile(4 + j)

    # ---------------- phase C: attention ----------------
    # Head pairs: head A on PE row strip 0, head B on strip 64; their score
    # pieces live in the two banks of one [128,1024] psum tile, so the two
    # row-packed matmuls run concurrently and one exp covers both heads.
    for hp in range(4):
        qt = qkT[hp]
        kt = qkT[4 + hp]
        for half in range(2):
            qlo = half * 1024
            nch = (qlo + 1024) // P
            # quarter-window PV psum per head: [65, 512], row 64 = denominator
            pv = {}
            for hh in range(2):
                for mq in range(2):
                    m = 2 * half + mq
                    pv[hh, m] = ps_pv.tile(
                        [P, 512], f32, name=f"pv_{hp}_{hh}_{m}", tag="ps_pv")
            ot = otp.tile([P, 1024], f32r, name=f"ot_{hp}_{half}", tag="ot")

            def normalize(hh, m):
                rc = rcp.tile([1, 512], f32, name=f"rc_{hp}_{hh}_{m}", tag="rc")
                nc.vector.reciprocal(rc[:], pv[hh, m][HD:HD + 1, :])
                rcb = rcp.tile([HD, 512], f32, name=f"rcb_{hp}_{hh}_{m}", tag="rcb")
                nc.gpsimd.partition_broadcast(rcb[:], rc[:])
                nc.vector.tensor_mul(
                    ot[hh * HD:(hh + 1) * HD, (m - 2 * half) * 512:(m - 2 * half + 1) * 512],
                    pv[hh, m][0:HD, :], rcb[:])

            for i in range(nch):
                for mq in range(2):
                    m = 2 * half + mq
                    ws = m * 512
                    s = max(i * P, ws)
                    if s >= ws + 512:
                        continue
                    o = s - ws
                    # head A piece in cols [o, 512), head B in [512, 1024-o)
                    # -> exp range [o, 1024-o) is contiguous and fully written
                    sc = ps_main.tile([P, 1024], f32, name=f"sc_{hp}_{i}_{m}",
                                      tag="main")
                    for hh in range(2):
                        r0 = hh * HD
                        c0 = o if hh == 0 else 512
                        nc.tensor.matmul(
                            sc[:, c0:c0 + 512 - o],
                            kt[r0:r0 + HD, i * P:(i + 1) * P],
                            qt[r0:r0 + HD, s:ws + 512],
                            start=True,
                            stop=True,
                        )
                    pt = ptp.tile([P, 1024], f32r, name=f"pt_{hp}_{i}_{m}",
                                  tag="pt")
                    nc.scalar.activation(pt[:, o:1024 - o], sc[:, o:1024 - o],
                                         EXP, scale=SCALE)
                    diag = ws <= i * P < ws + 512
                    for hh in range(2):
                        c0 = o if hh == 0 else 512
                        if diag:
                            nc.gpsimd.affine_select(
                                out=pt[:, c0:c0 + P],
                                in_=pt[:, c0:c0 + P],
                                compare_op=mybir.AluOpType.is_ge,
                                fill=0.0,
                                base=0,
                                pattern=[[1, P]],
                                channel_multiplier=-1,
                            )
                        nc.tensor.matmul(
                            pv[hh, m][0:HD + 1, o:],
                            vaug[i][:, 2 * hp + hh, :],
                            pt[:, c0:c0 + 512 - o],
                            start=(i == 0),
                            stop=(i == (ws + 512) // P - 1),
                        )
                # early normalize for the first quarter of the half
                if i == 8 * half + 3:
                    normalize(0, 2 * half)
                    normalize(1, 2 * half)
            normalize(0, 2 * half + 1)
            normalize(1, 2 * half + 1)
            nc.sync.dma_start(odram[hp * P:(hp + 1) * P, qlo:qlo + 1024], ot[:])

    stackAB.close()  # release x / w / xT pools

    # ---------------- phase D: proj ----------------
    stackD = contextlib.ExitStack()
    orp = stackD.enter_context(tc.tile_pool(name="orp", bufs=4))
    wpp = stackD.enter_context(tc.tile_pool(name="wpp", bufs=1))
    ysp = stackD.enter_context(tc.tile_pool(name="ysp", bufs=3))

    wp_t = wpp.tile([P, 4, C], f32r, name="wp", tag="wp")
    nc.sync.dma_start(wp_t[:], wp_d.rearrange("(a p) n -> p a n", p=P))
    oTr = []
    for a in range(4):
        o_t = orp.tile([P, T], f32r, name=f"oTr{a}", tag="oTr")
        nc.sync.dma_start(o_t[:], odram[a * P:(a + 1) * P, :])
        oTr.append(o_t)
    for mt in range(NCHUNK):  # cout tiles
        for twp in range(2):
            ps = ps_main.tile([P, 1024], f32, name=f"yps_{mt}_{twp}", tag="main")
            for a in range(4):
                for sw in range(2):
                    nc.tensor.matmul(
                        ps[:, sw * 512:(sw + 1) * 512],
                        wp_t[:, a, mt * P:(mt + 1) * P],
                        oTr[a][:, twp * 1024 + sw * 512:twp * 1024 + (sw + 1) * 512],
                        start=(a == 0),
                        stop=(a == 3),
                    )
            ys = ysp.tile([P, 1024], f32, name=f"ys_{mt}_{twp}", tag="ys")
            nc.scalar.copy(ys[:], ps[:])
            nc.sync.dma_start(
                yT_d[mt * P:(mt + 1) * P, twp * 1024:(twp + 1) * 1024], ys[:]
            )
    stackD.close()


def _build_program():
    import contextlib

    import concourse.bass as bass
    import concourse.mybir as mybir
    import concourse.tile as tile
    from concourse import bacc

    nc = bacc.Bacc("TRN2", target_bir_lowering=False, debug=False, num_devices=8)
    f32 = mybir.dt.float32
    aps = {
        "x": nc.dram_tensor("x", [T, C], f32, kind="ExternalInput").ap(),
        "wqkv": nc.dram_tensor("wqkv", [C, 3 * GQ], mybir.dt.float32r, kind="ExternalInput").ap(),
        "bqk": nc.dram_tensor("bqk", [P, 8], f32, kind="ExternalInput").ap(),
        "bv": nc.dram_tensor("bv", [GQ], f32, kind="ExternalInput").ap(),
        "wp": nc.dram_tensor("wp", [GQ, C], mybir.dt.float32r, kind="ExternalInput").ap(),
        "yT": nc.dram_tensor("yT", [C, T], f32, kind="ExternalOutput").ap(),
    }
    with tile.TileContext(nc) as tc:
        with contextlib.ExitStack() as ctx:
            _emit(ctx, tc, aps, mybir, bass)
    nc.compile()
    return nc


def get_program():
    global _PROGRAM
    if _PROGRAM is None:
        _PROGRAM = _build_program()
    return _PROGRAM


def make_in_maps(x, w_qkv, b_qkv, w_proj):
    x = np.asarray(x, np.float32)
    w_qkv = np.asarray(w_qkv, np.float32)
    b_qkv = np.asarray(b_qkv, np.float32)
    w_proj = np.asarray(w_proj, np.float32)
    in_maps = []
    for c in range(8):
        b = c // 2
        g = c % 2
        q0 = g * GQ
        wq = w_qkv[:, q0:q0 + GQ]
        wk = w_qkv[:, C + q0:C + q0 + GQ]
        wv = w_qkv[:, 2 * C + q0:2 * C + q0 + GQ]
        wqkv = np.ascontiguousarray(np.concatenate([wq, wk, wv], axis=1))
        bq = b_qkv[q0:q0 + GQ]
        bk = b_qkv[C + q0:C + q0 + GQ]
        bqk = np.ascontiguousarray(np.concatenate([bq, bk]).reshape(8, P).T)
        bv = np.ascontiguousarray(b_qkv[2 * C + q0:2 * C + q0 + GQ])
        in_maps.append({
            "x": np.ascontiguousarray(x[b]),
            "wqkv": wqkv,
            "bqk": bqk,
            "bv": bv,
            "wp": np.ascontiguousarray(w_proj[q0:q0 + GQ, :]),
        })
    return in_maps


def combine_outputs(outs, b_proj):
    b_proj = np.asarray(b_proj, np.float32)
    y = np.empty((B, T, C), np.float32)
    for b in range(B):
        acc = outs[2 * b] + outs[2 * b + 1]  # [C, T]
        y[b] = acc.T + b_proj
    return y


def kernel(x, w_qkv, b_qkv, w_proj, b_proj, _trace=False):
    from concourse import bass_utils

    nc = get_program()
    in_maps = make_in_maps(x, w_qkv, b_qkv, w_proj)
    res = bass_utils.run_bass_kernel_spmd(
        nc, in_maps, core_ids=list(range(8)), trace=_trace
    )
    outs = [r["yT"] for r in res.results]
    y = combine_outputs(outs, b_proj)
    if _trace:
        return y, res
    return y


# revision 22
# speedup vs baseline: 1.2801x; 1.0097x over previous
"""Causal self-attention on 8 TRN2 NeuronCores.

Sharding: core c handles batch b = c//2 and head-group g = c%2 (8 of 16 heads).
Each core computes its partial y^T = w_proj[slice].T @ o^T (contraction over its
512 o-channels); the host sums the two partials per batch and adds b_proj.

Shapes (hardcoded): B=4, T=2048, C=1024, H=16, HD=64.
"""

import numpy as np

B, T, C, H = 4, 2048, 1024, 16
HD = C // H          # 64
G = 2                # head groups
NHL = H // G         # 8 heads per core
GQ = NHL * HD        # 512 channel slice per core
P = 128
NT = T // P          # 16 token tiles / k-chunks
NCHUNK = C // P      # 8 contraction chunks for qkv
SCALE = 1.0 / float(np.sqrt(HD))

_PROGRAM = None


def _emit(ctx, tc, aps, mybir, bass):
    import contextlib

    nc = tc.nc
    f32 = mybir.dt.float32
    f32r = mybir.dt.float32r
    bf16 = mybir.dt.bfloat16
    EXP = mybir.ActivationFunctionType.Exp

    x_d, wqkv_d, bqk_d, bv_d, wp_d, yT_d = (
        aps["x"], aps["wqkv"], aps["bqk"], aps["bv"], aps["wp"], aps["yT"],
    )

    # ---------------- pools ----------------
    const = ctx.enter_context(tc.tile_pool(name="const", bufs=1))
    dramp = ctx.enter_context(tc.tile_pool(name="dramp", bufs=1, space="DRAM"))
    # psum: main 2x[128,1024] (4 banks) + pv 4x[128,512] (4 banks)
    ps_main = ctx.enter_context(tc.tile_pool(name="ps_main", bufs=2, space="PSUM"))
    ps_pv = ctx.enter_context(tc.tile_pool(name="ps_pv", bufs=4, space="PSUM"))

    qkp = ctx.enter_context(tc.tile_pool(name="qkp", bufs=8))
    vap = ctx.enter_context(tc.tile_pool(name="vap", bufs=16))
    ptp = ctx.enter_context(tc.tile_pool(name="ptp", bufs=3))
    otp = ctx.enter_context(tc.tile_pool(name="otp", bufs=2))
    rcp = ctx.enter_context(tc.tile_pool(name="rcp", bufs=2))

    # constants
    identity = const.tile([P, P], f32)
    from concourse.masks import make_identity
    make_identity(nc, identity)
    bqk_sb = const.tile([P, 8], f32)
    nc.sync.dma_start(bqk_sb[:], bqk_d[:])
    bvb = const.tile([P, GQ], f32)
    nc.sync.dma_start(bvb[:], bv_d[None, :].to_broadcast((P, GQ)))
    ones8 = const.tile([P, NHL, 1], f32)
    nc.vector.memset(ones8[:], 1.0)

    odram = dramp.tile([GQ, T], f32r, space="DRAM")

    # ---------------- phase A: load x, build xT ----------------
    stackAB = contextlib.ExitStack()
    xTp = stackAB.enter_context(tc.tile_pool(name="xTp", bufs=8))
    wqkp = stackAB.enter_context(tc.tile_pool(name="wqkp", bufs=4))
    wvp = stackAB.enter_context(tc.tile_pool(name="wvp", bufs=1))
    stackA = contextlib.ExitStack()
    xp = stackA.enter_context(tc.tile_pool(name="xp", bufs=2))

    xT = []  # 8 tiles [128 c, 2048 t]
    for r in range(NCHUNK):
        t_ = xTp.tile([P, T], f32r, name=f"xT{r}", tag="xT")
        xT.append(t_)

    for tg in range(NT // 2):  # groups of 2 t-tiles
        xts = []
        for tt in range(2):
            t = 2 * tg + tt
            x_t = xp.tile([P, C], f32, name=f"x_{t}", tag="x")
            nc.sync.dma_start(x_t[:], x_d[t * P:(t + 1) * P, :])
            xts.append(x_t)
        for r in range(NCHUNK):
            tp = ps_main.tile([P, 256], f32, name=f"tp_{tg}_{r}", tag="main")
            for tt in range(2):
                nc.tensor.transpose(
                    tp[:, tt * P:(tt + 1) * P],
                    xts[tt][:, r * P:(r + 1) * P],
                    identity,
                )
            nc.vector.tensor_copy(xT[r][:, tg * 256:(tg + 1) * 256], tp[:])
    stackA.close()

    # ---------------- phase B: qkv ----------------
    qkT = []  # bf16 tiles [128 c', 2048 t]; 0..3 = qT, 4..7 = kT
    for ct in range(8):
        o_t = qkp.tile([P, T], bf16, name=f"qkT{ct}", tag="qkT")
        qkT.append(o_t)

    vaug = []  # [128 k, 8 heads, 65] per k-chunk (col 64 = ones for denom)
    for t in range(NT):
        va = vap.tile([P, NHL, HD + 1], f32r, name=f"vaug{t}", tag="vaug")
        nc.vector.tensor_copy(va[:, :, HD:HD + 1], ones8[:])
        vaug.append(va)

    wqkv_r = wqkv_d.rearrange("(a p) n -> p a n", p=P)  # [128, 8, 1536]

    def emit_qk_tile(ct):
        col0 = ct * P
        w_t = wqkp.tile([P, NCHUNK, P], f32r, name=f"wqk_{ct}", tag="wqk")
        nc.sync.dma_start(w_t[:], wqkv_r[:, :, col0:col0 + P])
        for twp in range(2):  # two [*,1024] psum tiles per output tile
            ps = ps_main.tile([P, 1024], f32, name=f"qkps_{ct}_{twp}", tag="main")
            for a in range(NCHUNK):
                for sw in range(2):
                    nc.tensor.matmul(
                        ps[:, sw * 512:(sw + 1) * 512],
                        w_t[:, a, :],
                        xT[a][:, twp * 1024 + sw * 512:twp * 1024 + (sw + 1) * 512],
                        start=(a == 0),
                        stop=(a == NCHUNK - 1),
                    )
            nc.vector.tensor_scalar_add(
                qkT[ct][:, twp * 1024:(twp + 1) * 1024], ps[:], bqk_sb[:, ct:ct + 1]
            )

    def emit_v():
        w_t = wvp.tile([P, NCHUNK, GQ], f32r, name="wv", tag="wv")
        nc.sync.dma_start(w_t[:], wqkv_r[:, :, 2 * GQ:3 * GQ])
        for t in range(NT):
            ps = ps_pv.tile([P, GQ], f32, name=f"vps_{t}", tag="ps_pv")
            for a in range(NCHUNK):
                nc.tensor.matmul(
                    ps[:],
                    xT[a][:, t * P:(t + 1) * P],
                    w_t[:, a, :],
                    start=(a == 0),
                    stop=(a == NCHUNK - 1),
                )
            nc.vector.tensor_add(
                vaug[t][:, :, 0:HD],
                ps[:].rearrange("p (h d) -> p h d", h=NHL),
                bvb[:].rearrange("p (h d) -> p h d", h=NHL),
            )

    # ---------------- phase C: attention ----------------
    # Head pairs: head A on PE row strip 0, head B on strip 64; their score
    # pieces live in the two banks of one [128,1024] psum tile, so the two
    # row-packed matmuls run concurrently and one exp covers both heads.
    def attn_pair(hp):
        qt = qkT[hp]
        kt = qkT[4 + hp]
        for half in range(2):
            qlo = half * 1024
            nch = (qlo + 1024) // P
            # quarter-window PV psum per head: [65, 512], row 64 = denominator
            pv = {}
            for hh in range(2):
                for mq in range(2):
                    m = 2 * half + mq
                    pv[hh, m] = ps_pv.tile(
                        [P, 512], f32, name=f"pv_{hp}_{hh}_{m}", tag="ps_pv")
            ot = otp.tile([P, 1024], f32r, name=f"ot_{hp}_{half}", tag="ot")

            def normalize(hh, m):
                # 1/d = exp(-ln d): both funcs live in the same ACT table set
                # as the softmax Exp, so no table swaps; d > 0 always.
                lt = rcp.tile([1, 512], f32, name=f"lt_{hp}_{hh}_{m}", tag="lt")
                nc.scalar.activation(lt[:], pv[hh, m][HD:HD + 1, :],
                                     mybir.ActivationFunctionType.Ln)
                rc = rcp.tile([1, 512], f32, name=f"rc_{hp}_{hh}_{m}", tag="rc")
                nc.scalar.activation(rc[:], lt[:],
                                     mybir.ActivationFunctionType.Exp,
                                     scale=-1.0)
                rcb = rcp.tile([HD, 512], f32, name=f"rcb_{hp}_{hh}_{m}", tag="rcb")
                nc.gpsimd.partition_broadcast(rcb[:], rc[:])
                nc.vector.tensor_mul(
                    ot[hh * HD:(hh + 1) * HD, (m - 2 * half) * 512:(m - 2 * half + 1) * 512],
                    pv[hh, m][0:HD, :], rcb[:])

            for i in range(nch):
                for mq in range(2):
                    m = 2 * half + mq
                    ws = m * 512
                    s = max(i * P, ws)
                    if s >= ws + 512:
                        continue
                    o = s - ws
                    # head A piece in cols [o, 512), head B in [512, 1024-o)
                    # -> exp range [o, 1024-o) is contiguous and fully written
                    sc = ps_main.tile([P, 1024], f32, name=f"sc_{hp}_{i}_{m}",
                                      tag="main")
                    for hh in range(2):
                        r0 = hh * HD
                        c0 = o if hh == 0 else 512
                        nc.tensor.matmul(
                            sc[:, c0:c0 + 512 - o],
                            kt[r0:r0 + HD, i * P:(i + 1) * P],
                            qt[r0:r0 + HD, s:ws + 512],
                            start=True,
                            stop=True,
                        )
                    pt = ptp.tile([P, 1024], f32r, name=f"pt_{hp}_{i}_{m}",
                                  tag="pt")
                    nc.scalar.activation(pt[:, o:1024 - o], sc[:, o:1024 - o],
                                         EXP, scale=SCALE)
                    diag = ws <= i * P < ws + 512
                    for hh in range(2):
                        c0 = o if hh == 0 else 512
                        if diag:
                            nc.gpsimd.affine_select(
                                out=pt[:, c0:c0 + P],
                                in_=pt[:, c0:c0 + P],
                                compare_op=mybir.AluOpType.is_ge,
                                fill=0.0,
                                base=0,
                                pattern=[[1, P]],
                                channel_multiplier=-1,
                            )
                        nc.tensor.matmul(
                            pv[hh, m][0:HD + 1, o:],
                            vaug[i][:, 2 * hp + hh, :],
                            pt[:, c0:c0 + 512 - o],
                            start=(i == 0),
                            stop=(i == (ws + 512) // P - 1),
                        )
                # early normalize for the first quarter of the half
                if i == 8 * half + 3:
                    normalize(0, 2 * half)
                    normalize(1, 2 * half)
            normalize(0, 2 * half + 1)
            normalize(1, 2 * half + 1)
            nc.sync.dma_start(odram[hp * P:(hp + 1) * P, qlo:qlo + 1024], ot[:])

    # interleave qkv tile production with attention so the PE stream stays
    # dense through the ACT-bound attention stretches (keeps HAM warm)
    emit_qk_tile(0)
    emit_qk_tile(4)
    emit_v()
    attn_pair(0)
    for j in range(1, 4):
        emit_qk_tile(j)
        emit_qk_tile(4 + j)
        attn_pair(j)

    stackAB.close()  # release x / w / xT pools

    # ---------------- phase D: proj ----------------
    stackD = contextlib.ExitStack()
    orp = stackD.enter_context(tc.tile_pool(name="orp", bufs=4))
    wpp = stackD.enter_context(tc.tile_pool(name="wpp", bufs=1))
    ysp = stackD.enter_context(tc.tile_pool(name="ysp", bufs=3))

    wp_t = wpp.tile([P, 4, C], f32r, name="wp", tag="wp")
    nc.sync.dma_start(wp_t[:], wp_d.rearrange("(a p) n -> p a n", p=P))
    oTr = []
    for a in range(4):
        o_t = orp.tile([P, T], f32r, name=f"oTr{a}", tag="oTr")
        nc.sync.dma_start(o_t[:], odram[a * P:(a + 1) * P, :])
        oTr.append(o_t)
    for mt in range(NCHUNK):  # cout tiles
        for twp in range(2):
            ps = ps_main.tile([P, 1024], f32, name=f"yps_{mt}_{twp}", tag="main")
            for a in range(4):
                for sw in range(2):
                    nc.tensor.matmul(
                        ps[:, sw * 512:(sw + 1) * 512],
                        wp_t[:, a, mt * P:(mt + 1) * P],
                        oTr[a][:, twp * 1024 + sw * 512:twp * 1024 + (sw + 1) * 512],
                        start=(a == 0),
                        stop=(a == 3),
                    )
            ys = ysp.tile([P, 1024], f32, name=f"ys_{mt}_{twp}", tag="ys")
            nc.scalar.copy(ys[:], ps[:])
            nc.sync.dma_start(
                yT_d[mt * P:(mt + 1) * P, twp * 1024:(twp + 1) * 1024], ys[:]
            )
    stackD.close()


def _build_program():
    import contextlib

    import concourse.bass as bass
    import concourse.mybir as mybir
    import concourse.tile as tile
    from concourse import bacc

    nc = bacc.Bacc("TRN2", target_bir_lowering=False, debug=False, num_devices=8)
    f32 = mybir.dt.float32
    aps = {
        "x": nc.dram_tensor("x", [T, C], f32, kind="ExternalInput").ap(),
        "wqkv": nc.dram_tensor("wqkv", [C, 3 * GQ], mybir.dt.float32r, kind="ExternalInput").ap(),
        "bqk": nc.dram_tensor("bqk", [P, 8], f32, kind="ExternalInput").ap(),
        "bv": nc.dram_tensor("bv", [GQ], f32, kind="ExternalInput").ap(),
        "wp": nc.dram_tensor("wp", [GQ, C], mybir.dt.float32r, kind="ExternalInput").ap(),
        "yT": nc.dram_tensor("yT", [C, T], f32, kind="ExternalOutput").ap(),
    }
    with tile.TileContext(nc) as tc:
        with contextlib.ExitStack() as ctx:
            _emit(ctx, tc, aps, mybir, bass)
    nc.compile()
    return nc


def get_program():
    global _PROGRAM
    if _PROGRAM is None:
        _PROGRAM = _build_program()
    return _PROGRAM


def make_in_maps(x, w_qkv, b_qkv, w_proj):
    x = np.asarray(x, np.float32)
    w_qkv = np.asarray(w_qkv, np.float32)
    b_qkv = np.asarray(b_qkv, np.float32)
    w_proj = np.asarray(w_proj, np.float32)
    in_maps = []
    for c in range(8):
        b = c // 2
        g = c % 2
        q0 = g * GQ
        wq = w_qkv[:, q0:q0 + GQ]
        wk = w_qkv[:, C + q0:C + q0 + GQ]
        wv = w_qkv[:, 2 * C + q0:2 * C + q0 + GQ]
        wqkv = np.ascontiguousarray(np.concatenate([wq, wk, wv], axis=1))
        bq = b_qkv[q0:q0 + GQ]
        bk = b_qkv[C + q0:C + q0 + GQ]
        bqk = np.ascontiguousarray(np.concatenate([bq, bk]).reshape(8, P).T)
        bv = np.ascontiguousarray(b_qkv[2 * C + q0:2 * C + q0 + GQ])
        in_maps.append({
            "x": np.ascontiguousarray(x[b]),
            "wqkv": wqkv,
            "bqk": bqk,
            "bv": bv,
            "wp": np.ascontiguousarray(w_proj[q0:q0 + GQ, :]),
        })
    return in_maps


def combine_outputs(outs, b_proj):
    b_proj = np.asarray(b_proj, np.float32)
    y = np.empty((B, T, C), np.float32)
    for b in range(B):
        acc = outs[2 * b] + outs[2 * b + 1]  # [C, T]
        y[b] = acc.T + b_proj
    return y


def kernel(x, w_qkv, b_qkv, w_proj, b_proj, _trace=False):
    from concourse import bass_utils

    nc = get_program()
    in_maps = make_in_maps(x, w_qkv, b_qkv, w_proj)
    res = bass_utils.run_bass_kernel_spmd(
        nc, in_maps, core_ids=list(range(8)), trace=_trace
    )
    outs = [r["yT"] for r in res.results]
    y = combine_outputs(outs, b_proj)
    if _trace:
        return y, res
    return y


# revision 23
# speedup vs baseline: 1.5116x; 1.1809x over previous
"""Causal self-attention on 8 TRN2 NeuronCores.

Sharding: core c handles batch b = c//2 and head-group g = c%2 (8 of 16 heads).
Each core computes its partial y^T = w_proj[slice].T @ o^T (contraction over its
512 o-channels); the host sums the two partials per batch and adds b_proj.

Shapes (hardcoded): B=4, T=2048, C=1024, H=16, HD=64.
"""

import numpy as np

B, T, C, H = 4, 2048, 1024, 16
HD = C // H          # 64
G = 2                # head groups
NHL = H // G         # 8 heads per core
GQ = NHL * HD        # 512 channel slice per core
P = 128
NT = T // P          # 16 token tiles / k-chunks
NCHUNK = C // P      # 8 contraction chunks for qkv
SCALE = 1.0 / float(np.sqrt(HD))

_PROGRAM = None


def _emit(ctx, tc, aps, mybir, bass):
    import contextlib

    nc = tc.nc
    f32 = mybir.dt.float32
    f32r = mybir.dt.float32r
    bf16 = mybir.dt.bfloat16
    EXP = mybir.ActivationFunctionType.Exp

    x_d, wqkv_d, bqk_d, bv_d, wp_d, yT_d = (
        aps["x"], aps["wqkv"], aps["bqk"], aps["bv"], aps["wp"], aps["yT"],
    )

    # ---------------- pools ----------------
    const = ctx.enter_context(tc.tile_pool(name="const", bufs=1))
    dramp = ctx.enter_context(tc.tile_pool(name="dramp", bufs=1, space="DRAM"))
    # psum: main 2x[128,1024] (4 banks) + pv 4x[128,512] (4 banks)
    ps_main = ctx.enter_context(tc.tile_pool(name="ps_main", bufs=2, space="PSUM"))
    ps_pv = ctx.enter_context(tc.tile_pool(name="ps_pv", bufs=4, space="PSUM"))

    qkp = ctx.enter_context(tc.tile_pool(name="qkp", bufs=8))
    vap = ctx.enter_context(tc.tile_pool(name="vap", bufs=16))
    ptp = ctx.enter_context(tc.tile_pool(name="ptp", bufs=3))
    otp = ctx.enter_context(tc.tile_pool(name="otp", bufs=2))
    rcp = ctx.enter_context(tc.tile_pool(name="rcp", bufs=2))

    # constants
    identity = const.tile([P, P], f32)
    from concourse.masks import make_identity
    make_identity(nc, identity)
    bqk_sb = const.tile([P, 8], f32)
    nc.sync.dma_start(bqk_sb[:], bqk_d[:])
    bvb = const.tile([P, GQ], f32)
    nc.sync.dma_start(bvb[:], bv_d[None, :].to_broadcast((P, GQ)))
    ones8 = const.tile([P, NHL, 1], f32)
    nc.vector.memset(ones8[:], 1.0)

    odram = dramp.tile([GQ, T], f32r, space="DRAM")

    # ---------------- phase A: load x, build xT ----------------
    stackAB = contextlib.ExitStack()
    xTp = stackAB.enter_context(tc.tile_pool(name="xTp", bufs=8))
    wqkp = stackAB.enter_context(tc.tile_pool(name="wqkp", bufs=4))
    wvp = stackAB.enter_context(tc.tile_pool(name="wvp", bufs=1))
    stackA = contextlib.ExitStack()
    xp = stackA.enter_context(tc.tile_pool(name="xp", bufs=2))

    xT = []  # 8 tiles [128 c, 2048 t]
    for r in range(NCHUNK):
        t_ = xTp.tile([P, T], f32r, name=f"xT{r}", tag="xT")
        xT.append(t_)

    for tg in range(NT // 2):  # groups of 2 t-tiles
        xts = []
        for tt in range(2):
            t = 2 * tg + tt
            x_t = xp.tile([P, C], f32, name=f"x_{t}", tag="x")
            nc.sync.dma_start(x_t[:], x_d[t * P:(t + 1) * P, :])
            xts.append(x_t)
        for r in range(NCHUNK):
            tp = ps_main.tile([P, 256], f32, name=f"tp_{tg}_{r}", tag="main")
            for tt in range(2):
                nc.tensor.transpose(
                    tp[:, tt * P:(tt + 1) * P],
                    xts[tt][:, r * P:(r + 1) * P],
                    identity,
                )
            nc.vector.tensor_copy(xT[r][:, tg * 256:(tg + 1) * 256], tp[:])
    stackA.close()

    # ---------------- phase B: qkv ----------------
    qkT = []  # bf16 tiles [128 c', 2048 t]; 0..3 = qT, 4..7 = kT
    for ct in range(8):
        o_t = qkp.tile([P, T], bf16, name=f"qkT{ct}", tag="qkT")
        qkT.append(o_t)

    vaug = []  # [128 k, 8 heads, 65] per k-chunk (col 64 = ones for denom)
    for t in range(NT):
        va = vap.tile([P, NHL, HD + 1], f32r, name=f"vaug{t}", tag="vaug")
        nc.vector.tensor_copy(va[:, :, HD:HD + 1], ones8[:])
        vaug.append(va)

    wqkv_r = wqkv_d.rearrange("(a p) n -> p a n", p=P)  # [128, 8, 1536]

    def emit_qk_tile(ct):
        col0 = ct * P
        w_t = wqkp.tile([P, NCHUNK, P], f32r, name=f"wqk_{ct}", tag="wqk")
        nc.sync.dma_start(w_t[:], wqkv_r[:, :, col0:col0 + P])
        for twp in range(2):  # two [*,1024] psum tiles per output tile
            ps = ps_main.tile([P, 1024], f32, name=f"qkps_{ct}_{twp}", tag="main")
            for a in range(NCHUNK):
                for sw in range(2):
                    nc.tensor.matmul(
                        ps[:, sw * 512:(sw + 1) * 512],
                        w_t[:, a, :],
                        xT[a][:, twp * 1024 + sw * 512:twp * 1024 + (sw + 1) * 512],
                        start=(a == 0),
                        stop=(a == NCHUNK - 1),
                    )
            nc.vector.tensor_scalar_add(
                qkT[ct][:, twp * 1024:(twp + 1) * 1024], ps[:], bqk_sb[:, ct:ct + 1]
            )

    def emit_v():
        w_t = wvp.tile([P, NCHUNK, GQ], f32r, name="wv", tag="wv")
        nc.sync.dma_start(w_t[:], wqkv_r[:, :, 2 * GQ:3 * GQ])
        for t in range(NT):
            ps = ps_pv.tile([P, GQ], f32, name=f"vps_{t}", tag="ps_pv")
            for a in range(NCHUNK):
                nc.tensor.matmul(
                    ps[:],
                    xT[a][:, t * P:(t + 1) * P],
                    w_t[:, a, :],
                    start=(a == 0),
                    stop=(a == NCHUNK - 1),
                )
            nc.vector.tensor_add(
                vaug[t][:, :, 0:HD],
                ps[:].rearrange("p (h d) -> p h d", h=NHL),
                bvb[:].rearrange("p (h d) -> p h d", h=NHL),
            )

    # ---------------- phase C: attention ----------------
    # Head pairs: head A on PE row strip 0, head B on strip 64; score pieces
    # for the two heads live in the two banks of one [128,1024] psum tile, so
    # the row-packed matmuls run concurrently and one exp covers both heads.
    # Quarter-outer loop: each 512-wide q-window accumulates PV fully, then
    # normalizes while the next window runs (pv pool rotation hides it).
    def attn_pair(hp):
        qt = qkT[hp]
        kt = qkT[4 + hp]
        for m in range(4):  # quarter windows of 512 q
            ws = m * 512
            pvt = {}
            for hh in range(2):
                pvt[hh] = ps_pv.tile(
                    [P, 512], f32, name=f"pv_{hp}_{m}_{hh}", tag="ps_pv")
            for i in range(4 * m + 4):  # causal k-chunks for this window
                s = max(i * P, ws)
                o = s - ws
                # head A piece in cols [o, 512), head B in [512, 1024-o)
                sc = ps_main.tile([P, 1024], f32, name=f"sc_{hp}_{m}_{i}",
                                  tag="main")
                for hh in range(2):
                    r0 = hh * HD
                    c0 = o if hh == 0 else 512
                    nc.tensor.matmul(
                        sc[:, c0:c0 + 512 - o],
                        kt[r0:r0 + HD, i * P:(i + 1) * P],
                        qt[r0:r0 + HD, s:ws + 512],
                        start=True,
                        stop=True,
                    )
                pt = ptp.tile([P, 1024], f32r, name=f"pt_{hp}_{m}_{i}",
                              tag="pt")
                nc.scalar.activation(pt[:, o:1024 - o], sc[:, o:1024 - o],
                                     EXP, scale=SCALE)
                diag = i * P >= ws
                for hh in range(2):
                    c0 = o if hh == 0 else 512
                    if diag:
                        nc.gpsimd.affine_select(
                            out=pt[:, c0:c0 + P],
                            in_=pt[:, c0:c0 + P],
                            compare_op=mybir.AluOpType.is_ge,
                            fill=0.0,
                            base=0,
                            pattern=[[1, P]],
                            channel_multiplier=-1,
                        )
                    nc.tensor.matmul(
                        pvt[hh][0:HD + 1, o:],
                        vaug[i][:, 2 * hp + hh, :],
                        pt[:, c0:c0 + 512 - o],
                        start=(i == 0),
                        stop=(i == 4 * m + 3),
                    )
            # normalize both heads: ot rows 0:64 = head A, 64:128 = head B
            ot = otp.tile([P, 512], f32r, name=f"ot_{hp}_{m}", tag="ot")
            for hh in range(2):
                rc = rcp.tile([1, 512], f32, name=f"rc_{hp}_{m}_{hh}", tag="rc")
                nc.vector.reciprocal(rc[:], pvt[hh][HD:HD + 1, :])
                rcb = rcp.tile([HD, 512], f32, name=f"rcb_{hp}_{m}_{hh}",
                               tag="rcb")
                nc.gpsimd.partition_broadcast(rcb[:], rc[:])
                nc.vector.tensor_mul(
                    ot[hh * HD:(hh + 1) * HD, :], pvt[hh][0:HD, :], rcb[:])
            nc.sync.dma_start(odram[hp * P:(hp + 1) * P, ws:ws + 512], ot[:])

    # interleave qkv tile production with attention so the PE stream stays
    # dense through the ACT-bound attention stretches (keeps HAM warm)
    emit_qk_tile(0)
    emit_qk_tile(4)
    emit_v()
    attn_pair(0)
    for j in range(1, 4):
        emit_qk_tile(j)
        emit_qk_tile(4 + j)
        attn_pair(j)

    stackAB.close()  # release x / w / xT pools

    # ---------------- phase D: proj ----------------
    stackD = contextlib.ExitStack()
    orp = stackD.enter_context(tc.tile_pool(name="orp", bufs=4))
    wpp = stackD.enter_context(tc.tile_pool(name="wpp", bufs=1))
    ysp = stackD.enter_context(tc.tile_pool(name="ysp", bufs=3))

    wp_t = wpp.tile([P, 4, C], f32r, name="wp", tag="wp")
    nc.sync.dma_start(wp_t[:], wp_d.rearrange("(a p) n -> p a n", p=P))
    oTr = []
    for a in range(4):
        o_t = orp.tile([P, T], f32r, name=f"oTr{a}", tag="oTr")
        nc.sync.dma_start(o_t[:], odram[a * P:(a + 1) * P, :])
        oTr.append(o_t)
    for mt in range(NCHUNK):  # cout tiles
        for twp in range(2):
            ps = ps_main.tile([P, 1024], f32, name=f"yps_{mt}_{twp}", tag="main")
            for a in range(4):
                for sw in range(2):
                    nc.tensor.matmul(
                        ps[:, sw * 512:(sw + 1) * 512],
                        wp_t[:, a, mt * P:(mt + 1) * P],
                        oTr[a][:, twp * 1024 + sw * 512:twp * 1024 + (sw + 1) * 512],
                        start=(a == 0),
                        stop=(a == 3),
                    )
            ys = ysp.tile([P, 1024], f32, name=f"ys_{mt}_{twp}", tag="ys")
            nc.scalar.copy(ys[:], ps[:])
            nc.sync.dma_start(
                yT_d[mt * P:(mt + 1) * P, twp * 1024:(twp + 1) * 1024], ys[:]
            )
    stackD.close()


def _build_program():
    import contextlib

    import concourse.bass as bass
    import concourse.mybir as mybir
    import concourse.tile as tile
    from concourse import bacc

    nc = bacc.Bacc("TRN2", target_bir_lowering=False, debug=False, num_devices=8)
    f32 = mybir.dt.float32
    aps = {
        "x": nc.dram_tensor("x", [T, C], f32, kind="ExternalInput").ap(),
        "wqkv": nc.dram_tensor("wqkv", [C, 3 * GQ], mybir.dt.float32r, kind="ExternalInput").ap(),
        "bqk": nc.dram_tensor("bqk", [P, 8], f32, kind="ExternalInput").ap(),
        "bv": nc.dram_tensor("bv", [GQ], f32, kind="ExternalInput").ap(),
        "wp": nc.dram_tensor("wp", [GQ, C], mybir.dt.float32r, kind="ExternalInput").ap(),
        "yT": nc.dram_tensor("yT", [C, T], f32, kind="ExternalOutput").ap(),
    }
    with tile.TileContext(nc) as tc:
        with contextlib.ExitStack() as ctx:
            _emit(ctx, tc, aps, mybir, bass)
    nc.compile()
    return nc


def get_program():
    global _PROGRAM
    if _PROGRAM is None:
        _PROGRAM = _build_program()
    return _PROGRAM


def make_in_maps(x, w_qkv, b_qkv, w_proj):
    x = np.asarray(x, np.float32)
    w_qkv = np.asarray(w_qkv, np.float32)
    b_qkv = np.asarray(b_qkv, np.float32)
    w_proj = np.asarray(w_proj, np.float32)
    in_maps = []
    for c in range(8):
        b = c // 2
        g = c % 2
        q0 = g * GQ
        wq = w_qkv[:, q0:q0 + GQ]
        wk = w_qkv[:, C + q0:C + q0 + GQ]
        wv = w_qkv[:, 2 * C + q0:2 * C + q0 + GQ]
        wqkv = np.ascontiguousarray(np.concatenate([wq, wk, wv], axis=1))
        bq = b_qkv[q0:q0 + GQ]
        bk = b_qkv[C + q0:C + q0 + GQ]
        bqk = np.ascontiguousarray(np.concatenate([bq, bk]).reshape(8, P).T)
        bv = np.ascontiguousarray(b_qkv[2 * C + q0:2 * C + q0 + GQ])
        in_maps.append({
            "x": np.ascontiguousarray(x[b]),
            "wqkv": wqkv,
            "bqk": bqk,
            "bv": bv,
            "wp": np.ascontiguousarray(w_proj[q0:q0 + GQ, :]),
        })
    return in_maps


def combine_outputs(outs, b_proj):
    b_proj = np.asarray(b_proj, np.float32)
    y = np.empty((B, T, C), np.float32)
    for b in range(B):
        acc = outs[2 * b] + outs[2 * b + 1]  # [C, T]
        y[b] = acc.T + b_proj
    return y


def kernel(x, w_qkv, b_qkv, w_proj, b_proj, _trace=False):
    from concourse import bass_utils

    nc = get_program()
    in_maps = make_in_maps(x, w_qkv, b_qkv, w_proj)
    res = bass_utils.run_bass_kernel_spmd(
        nc, in_maps, core_ids=list(range(8)), trace=_trace
    )
    outs = [r["yT"] for r in res.results]
    y = combine_outputs(outs, b_proj)
    if _trace:
        return y, res
    return y
